# revision 20
# baseline (speedup 1.0000x reference)
"""Trainium2 Bass kernel for the GNN neighbor-aggregation module.

Computation (per row r with K=32 neighbors, D=64):
    scores[r,k]  = sum_d rel[r,k,d] * user[r%B, d]
    w[r,:]       = softmax(scores[r,:])                (no max-subtract; |score|<60)
    agg[r,d]     = (1/K) sum_k nei[r,k,d] * w[r,k] * norms[r,k]
    out[r,:]     = relu((self[r,:] + agg[r,:]) @ W.T + b)

Sharding: pure data parallelism over rows across 8 cores (8192 rows/core).
W, b and user_embeddings are replicated (user index = r mod 4096, and
8192 % 4096 == 0, so local user indexing is identical on every shard).

Raw-bass implementation (this walrus build rejects instructions carrying
embedded multi-sem waits, which rules out TileContext): explicit semaphores,
standalone wait_ge instructions, double-buffered SBUF/PSUM, per-engine
programs via nc.Block().

Per 128-row tile:
  DVE : scores = segmented-reduce(rel * user_bcast); softmax small ops;
        32x32 block-transpose of the weights; scatter into a block-diagonal
        [128,128] tile; x^T = agg^T + self^T.
  ACT : exp (with fused row-sum accumulator), PSUM->SBUF copies, relu+bias.
  PE  : 32 block-diagonal matmuls (lhsT = nei tile [128=(c,k), 64d],
        rhs = 4 block-diag weight columns) producing agg^T[d, r] directly;
        self^T via PE transpose; y^T = W^T.T @ x^T; transpose back.
  SP  : all DMAs (HWDGE).
"""

from contextlib import ExitStack

import numpy as np

import concourse.bass as bass
import concourse.mybir as mybir
from concourse.masks import make_identity

fp32 = mybir.dt.float32
AF = mybir.ActivationFunctionType
ALU = mybir.AluOpType

R, D, B, K = 65536, 64, 4096, 32
NCORES = 8
R_LOC = R // NCORES  # 8192 rows per core
P = 128              # rows per tile


def build_nc(r_loc: int = R_LOC):
    nc = bass.Bass("TRN2", target_bir_lowering=False, debug=False)

    self_d = nc.dram_tensor("self_v", [r_loc, D], fp32, kind="ExternalInput")
    nv_d = nc.dram_tensor("nei_v", [r_loc * K, D], fp32, kind="ExternalInput")
    nr_d = nc.dram_tensor("nei_r", [r_loc * K, D], fp32, kind="ExternalInput")
    nn_d = nc.dram_tensor("nei_n", [r_loc * K], fp32, kind="ExternalInput")
    user_d = nc.dram_tensor("user", [B, D], fp32, kind="ExternalInput")
    w_d = nc.dram_tensor("W", [D, D], fp32, kind="ExternalInput")
    b_d = nc.dram_tensor("b", [D], fp32, kind="ExternalInput")
    out_d = nc.dram_tensor("out", [r_loc, D], fp32, kind="ExternalOutput")

    T = r_loc // P  # number of 128-row tiles

    # DRAM views
    rel_rows = nr_d.ap().rearrange("(r k) d -> r (k d)", k=K)    # [r_loc, 2048]
    # nei matmul layout: nei[32c+k, 64j+d] = nv[(t*128+32c+j)*K + k, d],
    # one DMA per 32-partition block c with an affine [k, j, d] AP.
    nv_g = nv_d.ap().rearrange("(t c j k) d -> t c k j d", c=4, j=32, k=K)
    nn_rows = nn_d.ap().rearrange("(r k) -> r k", k=K)           # [r_loc, 32]

    a = nc.alloc_sbuf_tensor
    rel = [a(f"rel{i}", [P, K * D], fp32) for i in range(2)]
    nei = [a(f"nei{i}", [P, K * D], fp32) for i in range(2)]
    prod = a("prod", [P, K * D], fp32)
    user_t = [a(f"user{i}", [P, D], fp32) for i in range(2)]
    self_t = [a(f"self{i}", [P, D], fp32) for i in range(2)]
    norm_t = [a(f"norm{i}", [P, K], fp32) for i in range(2)]
    scores = [a(f"scores{i}", [P, K], fp32) for i in range(2)]
    e_t = [a(f"e{i}", [P, K], fp32) for i in range(2)]
    ssum = [a(f"ssum{i}", [P, 1], fp32) for i in range(2)]
    recip = a("recip", [P, 1], fp32)
    en = a("en", [P, K], fp32)
    w2 = a("w2", [P, K], fp32)
    vtd = a("vtd", [P, K], fp32)
    bdw = [a(f"bdw{i}", [P, P], fp32) for i in range(2)]
    self_s = [a(f"self_s{i}", [D, P], fp32) for i in range(2)]
    xt_s = [a(f"xt_s{i}", [D, P], fp32) for i in range(2)]
    yt_s = [a(f"yt_s{i}", [D, P], fp32) for i in range(2)]
    y_s = [a(f"y_s{i}", [P, D], fp32) for i in range(2)]
    wt = a("wt", [D, D], fp32)
    bias = a("bias", [D, 1], fp32)
    ident = a("ident", [P, P], fp32)

    ap = nc.alloc_psum_tensor
    agg_ps = [ap(f"agg{i}", [D, P], fp32) for i in range(2)]
    self_ps = [ap(f"selfT{i}", [D, P], fp32) for i in range(2)]
    yt_ps = [ap(f"yt{i}", [D, P], fp32) for i in range(2)]
    y_ps = [ap(f"y{i}", [P, D], fp32) for i in range(2)]

    s = nc.alloc_semaphore
    # DMA-completion sems are parity-split: two same-sem DMAs in flight can
    # interleave their 16 per-engine increments, so value 16 would not prove
    # the first transfer finished.  Parity buffers serialize same-sem use.
    s_rel = [s("s_rel0"), s("s_rel1")]
    s_nei = [s("s_nei0"), s("s_nei1")]
    s_user = [s("s_user0"), s("s_user1")]
    s_self = [s("s_self0"), s("s_self1")]
    s_norm = [s("s_norm0"), s("s_norm1")]
    s_out = [s("s_out0"), s("s_out1")]
    s_wtb, s_init = s("s_wtb"), s("s_init")
    s_scores, s_exp, s_bdw, s_xts = s("s_scores"), s("s_exp"), s("s_bdw"), s("s_xts")
    s_selfs, s_relu, s_ys = s("s_selfs"), s("s_relu"), s("s_ys")
    s_agg, s_wt, s_ytr = s("s_agg"), s("s_wt"), s("s_ytr")
    s_dve = s("s_dve")  # same-engine DVE RAW ordering (deep pipeline)

    def wge(eng, sem, val):
        if val > 0:
            eng.wait_ge(sem, val)

    with nc.Block() as block:

        @block.gpsimd
        def _(g):
            # GpSimd's 8 Q7 cores run ops out of order; sem-gate the
            # affine_select behind the memset it reads.
            g.memset(bdw[0].ap(), 0.0).then_inc(s_init, 1)
            g.memset(bdw[1].ap(), 0.0).then_inc(s_init, 1)
            g.memset(ident.ap(), 0.0).then_inc(s_init, 1)
            g.wait_ge(s_init, 3)
            g.affine_select(
                out=ident.ap(), in_=ident.ap(),
                compare_op=ALU.not_equal, fill=1.0, base=0,
                pattern=[[-1, P]], channel_multiplier=1,
            ).then_inc(s_init, 1)

        @block.sync
        def _(sp):
            with nc.allow_non_contiguous_dma(reason="16KB one-time W transpose load"):
                sp.dma_start(wt.ap(), w_d.ap().rearrange("j d -> d j")).then_inc(s_wtb, 16)
            sp.dma_start(bias.ap(), b_d.ap()[:, None]).then_inc(s_wtb, 16)
            for t in range(T):
                rb = t * P
                i = t % 2
                # buffer (t-2) consumers done?
                wge(sp, s_xts, t - 1)   # rel/user/norm consumed by DVE
                wge(sp, s_agg, t - 1)   # nei/self consumed by PE
                n = 16 * (t // 2 + 1)
                sp.dma_start(rel[i].ap(), rel_rows[rb:rb + P, :]).then_inc(s_rel[i], 16)
                for c in range(4):
                    sp.dma_start(
                        nei[i].ap()[32 * c:32 * c + 32, :].rearrange("p (j d) -> p j d", d=D),
                        nv_g[t, c],
                    ).then_inc(s_nei[i], 16)
                sp.dma_start(user_t[i].ap(), user_d.ap()[rb % B:rb % B + P, :]).then_inc(s_user[i], 16)
                sp.dma_start(self_t[i].ap(), self_d.ap()[rb:rb + P, :]).then_inc(s_self[i], 16)
                sp.dma_start(norm_t[i].ap(), nn_rows[rb:rb + P, :]).then_inc(s_norm[i], 16)
                # previous tile's output
                if t > 0:
                    wge(sp, s_ys, t)
                    sp.dma_start(out_d.ap()[rb - P:rb, :], y_s[(t - 1) % 2].ap()).then_inc(s_out[(t - 1) % 2], 16)
            wge(sp, s_ys, T)
            sp.dma_start(out_d.ap()[(T - 1) * P:T * P, :], y_s[(T - 1) % 2].ap()).then_inc(s_out[(T - 1) % 2], 16)

        @block.vector
        def _(v):
            wge(v, s_init, 4)
            for t in range(T):
                i = t % 2
                wge(v, s_exp, t - 1)          # scores/e/ssum buffer free
                wge(v, s_rel[i], 16 * (t // 2 + 1))
                wge(v, s_user[i], 16 * (t // 2 + 1))
                # s_dve orders same-engine RAW pairs (DVE pipeline overlap);
                # 5 increments per tile: TT, recip, en, w2, vtrans.
                wge(v, s_scores, t)           # reduce(t-1) done: prod WAR
                prod_v = prod.ap().rearrange("p (k d) -> p k d", k=K)
                nc.vector.tensor_tensor(
                    prod_v, rel[i].ap().rearrange("p (k d) -> p k d", k=K),
                    user_t[i].ap()[:, None, :].to_broadcast((P, K, D)), ALU.mult
                ).then_inc(s_dve, 1)
                wge(v, s_dve, 5 * t + 1)
                nc.vector.reduce_sum(
                    scores[i].ap(), prod_v, axis=mybir.AxisListType.X
                ).then_inc(s_scores, 1)
                wge(v, s_exp, t + 1)          # this tile's exp + row-sum done
                nc.vector.reciprocal(recip.ap(), ssum[i].ap()).then_inc(s_dve, 1)
                wge(v, s_norm[i], 16 * (t // 2 + 1))
                nc.vector.tensor_tensor(
                    en.ap(), e_t[i].ap(), norm_t[i].ap(), ALU.mult
                ).then_inc(s_dve, 1)
                wge(v, s_dve, 5 * t + 3)
                nc.vector.tensor_scalar(
                    w2.ap(), en.ap(), recip.ap(), 1.0 / K, ALU.mult, ALU.mult
                ).then_inc(s_dve, 1)
                wge(v, s_dve, 5 * t + 4)
                nc.vector.transpose(vtd.ap(), w2.ap()).then_inc(s_dve, 1)
                wge(v, s_agg, t - 1)          # bdw buffer free (PE of t-2 done)
                wge(v, s_dve, 5 * t + 5)
                for c in range(4):
                    inst = nc.vector.tensor_copy(
                        out=bdw[i].ap()[32 * c:32 * c + 32, 32 * c:32 * c + 32],
                        in_=vtd.ap()[32 * c:32 * c + 32, :])
                inst.then_inc(s_bdw, 1)
                wge(v, s_agg, t + 1)          # agg_ps + self_ps written by PE
                wge(v, s_selfs, t + 1)        # self_s copied by ACT
                nc.vector.tensor_tensor(
                    xt_s[i].ap(), agg_ps[i].ap(), self_s[i].ap(), ALU.add
                ).then_inc(s_xts, 1)

        @block.scalar
        def _(sc):
            wge(sc, s_wtb, 32)
            for t in range(T):
                i = t % 2
                wge(sc, s_xts, t - 1)         # e/ssum read by DVE of t-2 done
                wge(sc, s_scores, t + 1)
                nc.scalar.activation(
                    e_t[i].ap(), scores[i].ap(), AF.Exp, accum_out=ssum[i].ap()
                ).then_inc(s_exp, 1)
                wge(sc, s_agg, t + 1)         # selfT in PSUM
                nc.scalar.copy(self_s[i].ap(), self_ps[i].ap()).then_inc(s_selfs, 1)
                wge(sc, s_ytr, t - 1)         # yt_s read by PE of t-2 done
                wge(sc, s_wt, t + 1)
                nc.scalar.activation(
                    yt_s[i].ap(), yt_ps[i].ap(), AF.Relu, bias=bias.ap()
                ).then_inc(s_relu, 1)
                wge(sc, s_out[i], 16 * (t // 2))  # out-DMA of t-2 (same parity) done  # y_s consumed by out-DMA of t-2
                wge(sc, s_ytr, t + 1)
                nc.scalar.copy(y_s[i].ap(), y_ps[i].ap()).then_inc(s_ys, 1)

        @block.tensor
        def _(pe):
            wge(pe, s_init, 4)
            wge(pe, s_wtb, 32)
            for t in range(T):
                i = t % 2
                wge(pe, s_selfs, t - 1)       # self_ps read by ACT of t-2 done
                wge(pe, s_xts, t - 1)         # agg_ps read by DVE of t-2 done
                wge(pe, s_self[i], 16 * (t // 2 + 1))
                nc.tensor.matmul(self_ps[i].ap(), self_t[i].ap(), ident.ap(),
                                 is_transpose=True, start=True, stop=True)
                wge(pe, s_bdw, t + 1)
                wge(pe, s_nei[i], 64 * (t // 2 + 1))
                agg_v = agg_ps[i].ap().rearrange("d (c j) -> d j c", j=32)
                bdw_v = bdw[i].ap().rearrange("p (c j) -> p j c", j=32)
                for j in range(K):
                    inst = nc.tensor.matmul(
                        agg_v[:, j], nei[i].ap()[:, D * j:D * j + D],
                        bdw_v[:, j], start=True, stop=True)
                inst.then_inc(s_agg, 1)
                wge(pe, s_relu, t - 1)        # yt_ps read by ACT of t-2 done
                wge(pe, s_xts, t + 1)
                nc.tensor.matmul(yt_ps[i].ap(), wt.ap(), xt_s[i].ap(),
                                 start=True, stop=True).then_inc(s_wt, 1)
                wge(pe, s_ys, t - 1)          # y_ps read by ACT of t-2 done
                wge(pe, s_relu, t + 1)
                nc.tensor.matmul(y_ps[i].ap(), yt_s[i].ap(), ident.ap()[:D, :D],
                                 is_transpose=True, start=True, stop=True
                                 ).then_inc(s_ytr, 1)

    return nc


_NC_CACHE: dict = {}


def _get_nc(r_loc: int):
    if r_loc not in _NC_CACHE:
        _NC_CACHE[r_loc] = build_nc(r_loc)
    return _NC_CACHE[r_loc]


def kernel(self_vectors, neighbor_vectors, neighbor_relations, neighbor_norms,
           user_embeddings, W, b, trace: bool = False):
    from concourse.bass_utils import run_bass_kernel_spmd

    nc = _get_nc(R_LOC)
    nloc = R_LOC * K
    in_maps = []
    for s in range(NCORES):
        r0 = s * R_LOC
        n0 = r0 * K
        in_maps.append({
            "self_v": np.ascontiguousarray(self_vectors[r0:r0 + R_LOC]),
            "nei_v": np.ascontiguousarray(neighbor_vectors[n0:n0 + nloc]),
            "nei_r": np.ascontiguousarray(neighbor_relations[n0:n0 + nloc]),
            "nei_n": np.ascontiguousarray(neighbor_norms[n0:n0 + nloc]),
            "user": np.ascontiguousarray(user_embeddings),
            "W": np.ascontiguousarray(W),
            "b": np.ascontiguousarray(b),
        })
    res = run_bass_kernel_spmd(nc, in_maps, core_ids=list(range(NCORES)),
                               trace=trace)
    out = np.concatenate([res.results[s]["out"] for s in range(NCORES)], axis=0)
    if trace:
        return out, res
    return out


# revision 21
# speedup vs baseline: 1.8051x; 1.8051x over previous
"""Trainium2 Bass kernel for the GNN neighbor-aggregation module.

Computation (per row r with K=32 neighbors, D=64):
    scores[r,k]  = sum_d rel[r,k,d] * user[r%B, d]
    w[r,:]       = softmax(scores[r,:])                (no max-subtract; |score|<60)
    agg[r,d]     = (1/K) sum_k nei[r,k,d] * w[r,k] * norms[r,k]
    out[r,:]     = relu((self[r,:] + agg[r,:]) @ W.T + b)

Sharding: pure data parallelism over rows across 8 cores (8192 rows/core).
W, b and user_embeddings are replicated (user index = r mod 4096, and
8192 % 4096 == 0, so local user indexing is identical on every shard).

Raw-bass implementation (this walrus build rejects instructions carrying
embedded multi-sem waits, which rules out TileContext): explicit semaphores,
standalone wait_ge instructions, double-buffered SBUF/PSUM, per-engine
programs via nc.Block().

Per 128-row tile:
  DVE : scores = segmented-reduce(rel * user_bcast); softmax small ops;
        32x32 block-transpose of the weights; scatter into a block-diagonal
        [128,128] tile; x^T = agg^T + self^T.
  ACT : exp (with fused row-sum accumulator), PSUM->SBUF copies, relu+bias.
  PE  : 32 block-diagonal matmuls (lhsT = nei tile [128=(c,k), 64d],
        rhs = 4 block-diag weight columns) producing agg^T[d, r] directly;
        self^T via PE transpose; y^T = W^T.T @ x^T; transpose back.
  SP  : all DMAs (HWDGE).
"""

from contextlib import ExitStack

import numpy as np

import concourse.bass as bass
import concourse.mybir as mybir
from concourse.masks import make_identity

fp32 = mybir.dt.float32
bf16 = mybir.dt.bfloat16
AF = mybir.ActivationFunctionType
ALU = mybir.AluOpType

R, D, B, K = 65536, 64, 4096, 32
NCORES = 8
R_LOC = R // NCORES  # 8192 rows per core
P = 128              # rows per tile


def build_nc(r_loc: int = R_LOC):
    nc = bass.Bass("TRN2", target_bir_lowering=False, debug=False)

    self_d = nc.dram_tensor("self_v", [r_loc, D], fp32, kind="ExternalInput")
    nv_d = nc.dram_tensor("nei_v", [r_loc * K, D], bf16, kind="ExternalInput")
    nr_d = nc.dram_tensor("nei_r", [r_loc * K, D], bf16, kind="ExternalInput")
    nn_d = nc.dram_tensor("nei_n", [r_loc * K], fp32, kind="ExternalInput")
    user_d = nc.dram_tensor("user", [B, D], bf16, kind="ExternalInput")
    w_d = nc.dram_tensor("W", [D, D], fp32, kind="ExternalInput")
    b_d = nc.dram_tensor("b", [D], fp32, kind="ExternalInput")
    out_d = nc.dram_tensor("out", [r_loc, D], fp32, kind="ExternalOutput")

    T = r_loc // P  # number of 128-row tiles

    # DRAM views
    rel_rows = nr_d.ap().rearrange("(r k) d -> r (k d)", k=K)    # [r_loc, 2048]
    # nei matmul layout: nei[32c+k, 64j+d] = nv[(t*128+32c+j)*K + k, d],
    # one DMA per 32-partition block c with an affine [k, j, d] AP.
    nv_g = nv_d.ap().rearrange("(t c j k) d -> t c k j d", c=4, j=32, k=K)
    nn_rows = nn_d.ap().rearrange("(r k) -> r k", k=K)           # [r_loc, 32]

    a = nc.alloc_sbuf_tensor
    rel = [a(f"rel{i}", [P, K * D], bf16) for i in range(2)]
    nei = [a(f"nei{i}", [P, K * D], bf16) for i in range(2)]
    prod = a("prod", [P, K * D], bf16)
    user_t = [a(f"user{i}", [P, D], bf16) for i in range(2)]
    self_t = [a(f"self{i}", [P, D], fp32) for i in range(2)]
    norm_t = [a(f"norm{i}", [P, K], fp32) for i in range(2)]
    scores = [a(f"scores{i}", [P, K], fp32) for i in range(2)]
    e_t = [a(f"e{i}", [P, K], fp32) for i in range(2)]
    ssum = [a(f"ssum{i}", [P, 1], fp32) for i in range(2)]
    recip = a("recip", [P, 1], fp32)
    en = a("en", [P, K], fp32)
    w2 = a("w2", [P, K], fp32)
    vtd = a("vtd", [P, K], fp32)
    bdw = [a(f"bdw{i}", [P, P], bf16) for i in range(2)]
    self_s = [a(f"self_s{i}", [D, P], fp32) for i in range(2)]
    xt_s = [a(f"xt_s{i}", [D, P], fp32) for i in range(2)]
    yt_s = [a(f"yt_s{i}", [D, P], fp32) for i in range(2)]
    y_s = [a(f"y_s{i}", [P, D], fp32) for i in range(2)]
    wt = a("wt", [D, D], fp32)
    bias = a("bias", [D, 1], fp32)
    ident = a("ident", [P, P], fp32)

    ap = nc.alloc_psum_tensor
    agg_ps = [ap(f"agg{i}", [D, P], fp32) for i in range(2)]
    self_ps = [ap(f"selfT{i}", [D, P], fp32) for i in range(2)]
    yt_ps = [ap(f"yt{i}", [D, P], fp32) for i in range(2)]
    y_ps = [ap(f"y{i}", [P, D], fp32) for i in range(2)]

    s = nc.alloc_semaphore
    # DMA-completion sems are parity-split: two same-sem DMAs in flight can
    # interleave their 16 per-engine increments, so value 16 would not prove
    # the first transfer finished.  Parity buffers serialize same-sem use.
    s_rel = [s("s_rel0"), s("s_rel1")]
    s_nei = [s("s_nei0"), s("s_nei1")]
    s_user = [s("s_user0"), s("s_user1")]
    s_self = [s("s_self0"), s("s_self1")]
    s_norm = [s("s_norm0"), s("s_norm1")]
    s_out = [s("s_out0"), s("s_out1")]
    s_wtb, s_init = s("s_wtb"), s("s_init")
    s_scores, s_exp, s_bdw, s_xts = s("s_scores"), s("s_exp"), s("s_bdw"), s("s_xts")
    s_selfs, s_relu, s_ys = s("s_selfs"), s("s_relu"), s("s_ys")
    s_agg, s_wt, s_ytr = s("s_agg"), s("s_wt"), s("s_ytr")
    s_dve = s("s_dve")  # same-engine DVE RAW ordering (deep pipeline)

    def wge(eng, sem, val):
        if val > 0:
            eng.wait_ge(sem, val)

    with nc.Block() as block:

        @block.gpsimd
        def _(g):
            # GpSimd's 8 Q7 cores run ops out of order; sem-gate the
            # affine_select behind the memset it reads.
            g.memset(bdw[0].ap(), 0.0).then_inc(s_init, 1)
            g.memset(bdw[1].ap(), 0.0).then_inc(s_init, 1)
            g.memset(ident.ap(), 0.0).then_inc(s_init, 1)
            g.wait_ge(s_init, 3)
            g.affine_select(
                out=ident.ap(), in_=ident.ap(),
                compare_op=ALU.not_equal, fill=1.0, base=0,
                pattern=[[-1, P]], channel_multiplier=1,
            ).then_inc(s_init, 1)

        @block.sync
        def _(sp):
            with nc.allow_non_contiguous_dma(reason="16KB one-time W transpose load"):
                sp.dma_start(wt.ap(), w_d.ap().rearrange("j d -> d j")).then_inc(s_wtb, 16)
            sp.dma_start(bias.ap(), b_d.ap()[:, None]).then_inc(s_wtb, 16)
            for t in range(T):
                rb = t * P
                i = t % 2
                # buffer (t-2) consumers done?
                wge(sp, s_xts, t - 1)   # rel/user/norm consumed by DVE
                wge(sp, s_agg, t - 1)   # nei/self consumed by PE
                n = 16 * (t // 2 + 1)
                sp.dma_start(rel[i].ap(), rel_rows[rb:rb + P, :]).then_inc(s_rel[i], 16)
                for c in range(4):
                    sp.dma_start(
                        nei[i].ap()[32 * c:32 * c + 32, :].rearrange("p (j d) -> p j d", d=D),
                        nv_g[t, c],
                    ).then_inc(s_nei[i], 16)
                sp.dma_start(user_t[i].ap(), user_d.ap()[rb % B:rb % B + P, :]).then_inc(s_user[i], 16)
                sp.dma_start(self_t[i].ap(), self_d.ap()[rb:rb + P, :]).then_inc(s_self[i], 16)
                sp.dma_start(norm_t[i].ap(), nn_rows[rb:rb + P, :]).then_inc(s_norm[i], 16)
                # previous tile's output
                if t > 0:
                    wge(sp, s_ys, t)
                    sp.dma_start(out_d.ap()[rb - P:rb, :], y_s[(t - 1) % 2].ap()).then_inc(s_out[(t - 1) % 2], 16)
            wge(sp, s_ys, T)
            sp.dma_start(out_d.ap()[(T - 1) * P:T * P, :], y_s[(T - 1) % 2].ap()).then_inc(s_out[(T - 1) % 2], 16)

        @block.vector
        def _(v):
            wge(v, s_init, 4)
            for t in range(T):
                i = t % 2
                wge(v, s_exp, t - 1)          # scores/e/ssum buffer free
                wge(v, s_rel[i], 16 * (t // 2 + 1))
                wge(v, s_user[i], 16 * (t // 2 + 1))
                # s_dve orders same-engine RAW pairs (DVE pipeline overlap);
                # 5 increments per tile: TT, recip, en, w2, vtrans.
                wge(v, s_scores, t)           # reduce(t-1) done: prod WAR
                prod_v = prod.ap().rearrange("p (k d) -> p k d", k=K)
                nc.vector.tensor_tensor(
                    prod_v, rel[i].ap().rearrange("p (k d) -> p k d", k=K),
                    user_t[i].ap()[:, None, :].to_broadcast((P, K, D)), ALU.mult
                ).then_inc(s_dve, 1)
                wge(v, s_dve, 5 * t + 1)
                nc.vector.reduce_sum(
                    scores[i].ap(), prod_v, axis=mybir.AxisListType.X
                ).then_inc(s_scores, 1)
                wge(v, s_exp, t + 1)          # this tile's exp + row-sum done
                nc.vector.reciprocal(recip.ap(), ssum[i].ap()).then_inc(s_dve, 1)
                wge(v, s_norm[i], 16 * (t // 2 + 1))
                nc.vector.tensor_tensor(
                    en.ap(), e_t[i].ap(), norm_t[i].ap(), ALU.mult
                ).then_inc(s_dve, 1)
                wge(v, s_dve, 5 * t + 3)
                nc.vector.tensor_scalar(
                    w2.ap(), en.ap(), recip.ap(), 1.0 / K, ALU.mult, ALU.mult
                ).then_inc(s_dve, 1)
                wge(v, s_dve, 5 * t + 4)
                nc.vector.transpose(vtd.ap(), w2.ap()).then_inc(s_dve, 1)
                wge(v, s_agg, t - 1)          # bdw buffer free (PE of t-2 done)
                wge(v, s_dve, 5 * t + 5)
                for c in range(4):
                    inst = nc.vector.tensor_copy(
                        out=bdw[i].ap()[32 * c:32 * c + 32, 32 * c:32 * c + 32],
                        in_=vtd.ap()[32 * c:32 * c + 32, :])
                inst.then_inc(s_bdw, 1)
                wge(v, s_agg, t + 1)          # agg_ps + self_ps written by PE
                wge(v, s_selfs, t + 1)        # self_s copied by ACT
                nc.vector.tensor_tensor(
                    xt_s[i].ap(), agg_ps[i].ap(), self_s[i].ap(), ALU.add
                ).then_inc(s_xts, 1)

        @block.scalar
        def _(sc):
            wge(sc, s_wtb, 32)
            for t in range(T):
                i = t % 2
                wge(sc, s_xts, t - 1)         # e/ssum read by DVE of t-2 done
                wge(sc, s_scores, t + 1)
                nc.scalar.activation(
                    e_t[i].ap(), scores[i].ap(), AF.Exp, accum_out=ssum[i].ap()
                ).then_inc(s_exp, 1)
                wge(sc, s_agg, t + 1)         # selfT in PSUM
                nc.scalar.copy(self_s[i].ap(), self_ps[i].ap()).then_inc(s_selfs, 1)
                wge(sc, s_ytr, t - 1)         # yt_s read by PE of t-2 done
                wge(sc, s_wt, t + 1)
                nc.scalar.activation(
                    yt_s[i].ap(), yt_ps[i].ap(), AF.Relu, bias=bias.ap()
                ).then_inc(s_relu, 1)
                wge(sc, s_out[i], 16 * (t // 2))  # out-DMA of t-2 (same parity) done  # y_s consumed by out-DMA of t-2
                wge(sc, s_ytr, t + 1)
                nc.scalar.copy(y_s[i].ap(), y_ps[i].ap()).then_inc(s_ys, 1)

        @block.tensor
        def _(pe):
            wge(pe, s_init, 4)
            wge(pe, s_wtb, 32)
            for t in range(T):
                i = t % 2
                wge(pe, s_selfs, t - 1)       # self_ps read by ACT of t-2 done
                wge(pe, s_xts, t - 1)         # agg_ps read by DVE of t-2 done
                wge(pe, s_self[i], 16 * (t // 2 + 1))
                nc.tensor.matmul(self_ps[i].ap(), self_t[i].ap(), ident.ap(),
                                 is_transpose=True, start=True, stop=True)
                wge(pe, s_bdw, t + 1)
                wge(pe, s_nei[i], 64 * (t // 2 + 1))
                agg_v = agg_ps[i].ap().rearrange("d (c j) -> d j c", j=32)
                bdw_v = bdw[i].ap().rearrange("p (c j) -> p j c", j=32)
                for j in range(K):
                    inst = nc.tensor.matmul(
                        agg_v[:, j], nei[i].ap()[:, D * j:D * j + D],
                        bdw_v[:, j], start=True, stop=True)
                inst.then_inc(s_agg, 1)
                wge(pe, s_relu, t - 1)        # yt_ps read by ACT of t-2 done
                wge(pe, s_xts, t + 1)
                nc.tensor.matmul(yt_ps[i].ap(), wt.ap(), xt_s[i].ap(),
                                 start=True, stop=True).then_inc(s_wt, 1)
                wge(pe, s_ys, t - 1)          # y_ps read by ACT of t-2 done
                wge(pe, s_relu, t + 1)
                nc.tensor.matmul(y_ps[i].ap(), yt_s[i].ap(), ident.ap()[:D, :D],
                                 is_transpose=True, start=True, stop=True
                                 ).then_inc(s_ytr, 1)

    return nc


_NC_CACHE: dict = {}


def _get_nc(r_loc: int):
    if r_loc not in _NC_CACHE:
        _NC_CACHE[r_loc] = build_nc(r_loc)
    return _NC_CACHE[r_loc]


def kernel(self_vectors, neighbor_vectors, neighbor_relations, neighbor_norms,
           user_embeddings, W, b, trace: bool = False):
    from concourse.bass_utils import run_bass_kernel_spmd

    import ml_dtypes

    nc = _get_nc(R_LOC)
    nloc = R_LOC * K
    # bf16 inputs for the scores TT (DVE 2x mode) and the aggregation
    # matmuls (single-pass PE + FWL); halves their HBM traffic.  Verified
    # end-to-end rel_l2 impact: 1.1e-04.
    nv_bf = np.ascontiguousarray(neighbor_vectors).astype(ml_dtypes.bfloat16)
    nr_bf = np.ascontiguousarray(neighbor_relations).astype(ml_dtypes.bfloat16)
    user_bf = np.ascontiguousarray(user_embeddings).astype(ml_dtypes.bfloat16)
    in_maps = []
    for s in range(NCORES):
        r0 = s * R_LOC
        n0 = r0 * K
        in_maps.append({
            "self_v": np.ascontiguousarray(self_vectors[r0:r0 + R_LOC]),
            "nei_v": nv_bf[n0:n0 + nloc],
            "nei_r": nr_bf[n0:n0 + nloc],
            "nei_n": np.ascontiguousarray(neighbor_norms[n0:n0 + nloc]),
            "user": user_bf,
            "W": np.ascontiguousarray(W),
            "b": np.ascontiguousarray(b),
        })
    res = run_bass_kernel_spmd(nc, in_maps, core_ids=list(range(NCORES)),
                               trace=trace)
    out = np.concatenate([res.results[s]["out"] for s in range(NCORES)], axis=0)
    if trace:
        return out, res
    return out
